# revision 10
# baseline (speedup 1.0000x reference)
"""Trainium2 Bass kernel for nn_BilinearGrounding.

Reference computation:
    encI_p[b]  = encI[b] @ K_w.T + K_b                  # [100, 768]
    logits[b]  = encT[b] @ bil_w[0] @ encI_p[b].T       # [128, 100]
                 + bil_b[0] + mask[b, 0]

Kernel strategy (v4):
  * One-time weight fold on host:
        M = bil_w[0] @ K_w          [768, 2048]
        cterm[b,t] = encT[b,t,:] . (bil_w[0] @ K_b)
    so the device computes, per batch b:
        Y[b]      = M @ encI[b].T                       # [768, 100]
        logits[b] = encT[b] @ Y[b] + (mask[b] + bil_b + cterm[b])
  * Data-parallel over batch: 8 batches/core x 8 cores; bf16 activations.
  * Stage Y splits the OUTPUT-ROW (dc) dim into two phases: phase A
    computes Y rows dc0-2, phase B rows dc3-5. Each phase streams all 16
    i-chunks at the full 800 columns (as L/R 400-col matmul pairs, PSUM
    bank limit), keeping its 6 accumulators (3 dc x 2 halves) RESIDENT
    in PSUM for the whole contraction -- Y spills once per phase.
    Rationale vs column-split phases: the PE consumes a chunk in 2.02us
    here, so the DMA need-rate is only (98K mtb_lo + 205K enci)/2.02us
    = 150 B/ns, robust against the measured DMA realities: ~0.9us ring
    dead-time per transfer (forces big slabs) and whole-transfer wait
    granularity. mtb_hi prefetches entirely under phase A.
  * Stage C (48 MMs, ~44ns each, LDW hidden) runs at the end; phase B's
    last two chunks run dc-major so per-acc spills (alternating DVE/ACT)
    pipeline ahead of stage C's reads (batches 0-3 need only L-half
    spills, 4-7 only R-half). Stage-C PSUM groups are b-outer
    (whole-bank has_written safety); pc1/pc2/pc3 rotate over 2 banks so
    each epilogue reads one bank while the PE fills the other. Last
    store is a single batch on the otherwise-idle ACT ring.
  * ALL dram tensors ship per-partition-contiguous (row p = everything
    partition p receives, in tile order): one contiguous run per
    partition per transfer on both HBM and SBUF sides -- fast HWDGE
    descgen (strided slabs measured 2.4-6.7us descgen each).
  * Junk bf16 fillers bridge the PE HAM warmup from the preamble end
    (~7.2us) to the first data matmul (~11us).
"""

import numpy as np

B, N_TOK, N_ROI = 64, 128, 100
T_HID, I_HID = 768, 2048
NCORES = 8
NB = B // NCORES          # batches per core
NCOL = NB * N_ROI         # 800
NTCOL = NB * N_TOK        # 1024
IC = I_HID // 128         # 16 i-chunks (contraction for Y)
DC = T_HID // 128         # 6  d-chunks (contraction for logits)
HD = DC // 2              # 3 d-chunks per phase
HW = HD * 128             # 384 mtb cols per phase

FILLERS = 7
_CACHE = {}


def _build():
    import concourse.tile as tile
    from concourse import bacc, mybir
    from contextlib import ExitStack

    f32 = mybir.dt.float32
    bf16 = mybir.dt.bfloat16

    nc = bacc.Bacc("TRN2", target_bir_lowering=False)
    # all inputs per-partition-contiguous: row p = partition p's data
    d_mtbl = nc.dram_tensor("mtbl", [128, IC * HW], bf16,
                            kind="ExternalInput")
    d_mtbh = nc.dram_tensor("mtbh", [128, IC * HW], bf16,
                            kind="ExternalInput")
    d_enci = nc.dram_tensor("enci", [128, IC * NCOL], bf16,
                            kind="ExternalInput")
    d_enct = nc.dram_tensor("enct", [128, DC * NTCOL], bf16,
                            kind="ExternalInput")
    d_mask = nc.dram_tensor("maskb", [128, NCOL], f32, kind="ExternalInput")
    # output also per-partition-contiguous: row p = (b, r) for token p
    d_out = nc.dram_tensor("out", [128, NB * N_ROI], f32,
                           kind="ExternalOutput")

    mtbl_r = d_mtbl[:, :].rearrange("p (ic t) -> p ic t", ic=IC)
    mtbh_r = d_mtbh[:, :].rearrange("p (ic t) -> p ic t", ic=IC)
    enci_r = d_enci[:, :].rearrange("p (ic c) -> p ic c", ic=IC)
    enct_r = d_enct[:, :].rearrange("p (dc c) -> p dc c", dc=DC)
    out_r = d_out[:, :].rearrange("p (b r) -> p b r", b=NB)       # [128,8,100]

    with tile.TileContext(nc) as tc, ExitStack() as ctx:
        sb = ctx.enter_context(tc.tile_pool(name="sb", bufs=1))
        ps = ctx.enter_context(tc.tile_pool(name="ps", bufs=1, space="PSUM"))

        MTBL = sb.tile([128, IC, HW], bf16)       # M^T cols 0:384 (dc 0-2)
        MTBH = sb.tile([128, IC, HW], bf16)       # M^T cols 384:768 (dc 3-5)
        ENCI = sb.tile([128, IC, NCOL], bf16)     # encI^T chunks
        ENCT = sb.tile([128, DC, NTCOL], bf16)    # encT^T chunks (lhsT)
        MASK = sb.tile([128, NCOL], f32)          # mask + bil_b + encT.c
        Y = sb.tile([128, DC, NCOL], bf16)        # Y = M @ encI^T
        OUT = sb.tile([128, NB, N_ROI], f32)
        JUNK = sb.tile([128, 512], bf16)

        # ---- DMA triggers, big slabs, consumption order, THREE queues.
        # Ring dead time is ~0.9us per transfer and waits are whole-
        # transfer, so the early chunks (tight deadlines) spread across
        # SP + ACT + the gpsimd SWDGE queue to cut each queue's serial
        # latency chain. mtb_hi prefetches under phase A; mask/enct late.
        nc.gpsimd.memset(JUNK[:, :], 0.25)
        nc.sync.dma_start(out=MTBL[:, 0, :], in_=mtbl_r[:, 0, :])
        nc.sync.dma_start(out=MTBL[:, 1:4, :], in_=mtbl_r[:, 1:4, :])
        nc.sync.dma_start(out=MTBL[:, 4:8, :], in_=mtbl_r[:, 4:8, :])
        nc.sync.dma_start(out=MTBL[:, 8:IC, :], in_=mtbl_r[:, 8:IC, :])
        nc.sync.dma_start(out=MTBH[:, 0:8, :], in_=mtbh_r[:, 0:8, :])
        nc.sync.dma_start(out=MTBH[:, 8:IC, :], in_=mtbh_r[:, 8:IC, :])
        nc.sync.dma_start(out=MASK[:, :], in_=d_mask[:, :])
        nc.scalar.dma_start(out=ENCI[:, 0, :], in_=enci_r[:, 0, :])
        nc.scalar.dma_start(out=ENCI[:, 2:4, :], in_=enci_r[:, 2:4, :])
        nc.scalar.dma_start(out=ENCI[:, 6:10, :], in_=enci_r[:, 6:10, :])
        nc.scalar.dma_start(out=ENCI[:, 10:IC, :], in_=enci_r[:, 10:IC, :])
        nc.scalar.dma_start(out=ENCT[:, :, :], in_=enct_r[:, :, :])
        nc.gpsimd.dma_start(out=ENCI[:, 1:2, :], in_=enci_r[:, 1:2, :])
        nc.gpsimd.dma_start(out=ENCI[:, 4:6, :], in_=enci_r[:, 4:6, :])

        # ---- fillers bridge HAM warmup (no DMA deps)
        fp = ps.tile([128, 512], f32, tag="pc", bufs=2, name="fill")
        for i in range(FILLERS):
            nc.tensor.matmul(fp[:, :], JUNK[:, 0:128], JUNK[:, :],
                             start=(i == 0), stop=(i == FILLERS - 1))

        # 6 resident accumulators per phase: (dc%3) x (L/R half)
        def acc_tiles(ph):
            return [[ps.tile([128, 400], f32, tag=f"a{j}{h}", bufs=1,
                             name=f"acc{ph}_{j}{h}")
                     for h in range(2)] for j in range(HD)]

        def ymm(accs, mtb, ic, j, h, start, stop):
            nc.tensor.matmul(
                accs[j][h][:, :], mtb[:, ic, j * 128:(j + 1) * 128],
                ENCI[:, ic, h * 400:(h + 1) * 400], start=start, stop=stop)

        def spill(accs, ph, j, h):
            dc = ph * HD + j
            csl = slice(h * 400, (h + 1) * 400)
            if (j + h) % 2 == 0:
                nc.vector.tensor_copy(out=Y[:, dc, csl], in_=accs[j][h][:, :])
            else:
                nc.scalar.copy(out=Y[:, dc, csl], in_=accs[j][h][:, :])

        def phase(ph):
            mtb = MTBL if ph == 0 else MTBH
            accs = acc_tiles(ph)
            for ic in range(IC - 2):
                for j in range(HD):
                    for h in range(2):
                        ymm(accs, mtb, ic, j, h, start=(ic == 0), stop=False)
            # last two chunks dc-major so spills pipeline ahead of the
            # next phase / stage C
            for j in range(HD):
                for h in range(2):
                    ymm(accs, mtb, IC - 2, j, h, start=False, stop=False)
                for h in range(2):
                    ymm(accs, mtb, IC - 1, j, h, start=False, stop=True)
                for h in range(2):
                    spill(accs, ph, j, h)

        def stagec(pc, bb0, nb):
            # b-outer: each 100-col slice's start..stop group completes
            # before the next slice's start clears the bank's has_written
            for i in range(nb):
                b = bb0 + i
                for dc in range(DC):
                    nc.tensor.matmul(
                        pc[:, i * N_ROI:(i + 1) * N_ROI],
                        ENCT[:, dc, b * 128:(b + 1) * 128],
                        Y[:, dc, b * N_ROI:(b + 1) * N_ROI],
                        start=(dc == 0), stop=(dc == DC - 1))

        def epilogue(pc, b0, nb, store_eng):
            nc.vector.tensor_add(
                OUT[:, b0:b0 + nb, :], pc[:, :],
                MASK[:, b0 * N_ROI:(b0 + nb) * N_ROI])
            store_eng.dma_start(out=out_r[:, b0:b0 + nb, :],
                                in_=OUT[:, b0:b0 + nb, :])

        phase(0)
        phase(1)

        # ---- stage C: batches 0-3 | 4-6 | 7; last store smallest on ACT
        pc1 = ps.tile([128, 4 * N_ROI], f32, tag="pc", bufs=2, name="pc1")
        stagec(pc1, 0, 4)
        epilogue(pc1, 0, 4, nc.sync)
        pc2 = ps.tile([128, 3 * N_ROI], f32, tag="pc", bufs=2, name="pc2")
        stagec(pc2, 4, 3)
        epilogue(pc2, 4, 3, nc.sync)
        pc3 = ps.tile([128, N_ROI], f32, tag="pc", bufs=2, name="pc3")
        stagec(pc3, 7, 1)
        epilogue(pc3, 7, 1, nc.scalar)

    nc.finalize()
    return nc


def _get_nc():
    if "nc" not in _CACHE:
        _CACHE["nc"] = _build()
    return _CACHE["nc"]


def _pcontig(a, nchunk):
    """[nchunk*128, c] -> per-partition-contiguous [128, nchunk*c]."""
    n = a.shape[0] // nchunk
    return np.ascontiguousarray(
        a.reshape(nchunk, n, -1).transpose(1, 0, 2).reshape(n, -1))


def _prep_in_maps(encT, encI, mask, K_w, K_b, bil_w, bil_b):
    import ml_dtypes

    bf16 = ml_dtypes.bfloat16
    encT = np.asarray(encT, np.float32)
    encI = np.asarray(encI, np.float32)
    mask = np.asarray(mask, np.float32)
    K_w = np.asarray(K_w, np.float32)
    K_b = np.asarray(K_b, np.float32)
    bil_w = np.asarray(bil_w, np.float32)
    bil_b = np.asarray(bil_b, np.float32)

    # One-time weight fold (f64 for accuracy); folded weight ships as bf16
    M = bil_w[0].astype(np.float64) @ K_w.astype(np.float64)
    c = bil_w[0].astype(np.float64) @ K_b.astype(np.float64)
    mt = np.ascontiguousarray(M.T).astype(np.float32)             # [2048, 768]
    mtbl = _pcontig(mt[:, 0:HW], IC).astype(bf16)
    mtbh = _pcontig(mt[:, HW:], IC).astype(bf16)

    in_maps = []
    for cid in range(NCORES):
        sl = slice(cid * NB, (cid + 1) * NB)
        enci_t = encI[sl].transpose(2, 0, 1).reshape(I_HID, NCOL)
        enct_t = encT[sl].transpose(2, 0, 1).reshape(T_HID, NTCOL)
        enci = _pcontig(enci_t, IC).astype(bf16)
        enct = _pcontig(enct_t, DC).astype(bf16)
        # cterm[b,t] = encT[b,t,:] . c -- folded into the mask epilogue
        cterm = encT[sl].astype(np.float64) @ c                   # [8, 128]
        maskb = np.ascontiguousarray(
            (mask[sl, 0].transpose(1, 0, 2)                       # [128,8,100]
             + cterm.T[:, :, None]
             + np.float64(bil_b[0])).reshape(128, NCOL)).astype(np.float32)
        in_maps.append({"mtbl": mtbl, "mtbh": mtbh, "enci": enci,
                        "enct": enct, "maskb": maskb})
    return in_maps


def _run(inputs: dict, trace: bool = False, tmpdir=None):
    from concourse.bass_utils import run_bass_kernel_spmd

    in_maps = _prep_in_maps(**inputs)
    nc = _get_nc()
    res = run_bass_kernel_spmd(nc, in_maps, list(range(NCORES)), trace=trace,
                               tmpdir=tmpdir)
    out = np.concatenate(
        [res.results[i]["out"].reshape(N_TOK, NB, N_ROI).transpose(1, 0, 2)
         for i in range(NCORES)], axis=0)
    return out, res


def kernel(**inputs) -> np.ndarray:
    out, _ = _run(inputs, trace=False)
    return out
